# revision 5
# baseline (speedup 1.0000x reference)
"""Trainium2 Bass kernel for nn_BRASKModel (span-pairing + relation attention).

Contract: kernel(**inputs) takes FULL inputs (h_gs [8,768], embs [8,256,768],
params pytree) and returns the reference 10-tuple. Data-parallel over batch:
core c computes sentence c end-to-end on one NeuronCore.

Per-core device algorithm (validated against the jax reference in fp64/fp32):
  - head logits via TensorE; masks from logit sign (sigmoid(x)>0.5 <=> x>0)
  - greedy span pairing via closed form: exclusive/inclusive cumsums +
    prefix-max (hardware tensor_tensor_scan), pend via one-hot matmul
  - gather+scatter compaction as one-hot matmuls (TensorE)
  - attention energies tanh(Wr@rel + Wg@g + Wx@x) with the R-dim handled by
    per-partition-scalar adds (h on partitions), tanh on ScalarE,
    V-dot as M=1 matmuls accumulating over h-chunks in PSUM
  - softmax over L on [R, L] layout, transposed out via TensorE
"""
import numpy as np
from contextlib import ExitStack

B, L, H, R, TE = 8, 256, 768, 24, 100
HC = H // 128          # 6 h chunks
LT = L // 128          # 2 l tiles
RB = 2                 # r blocks
RPB = R // RB          # 12 r per block
SLC = RPB * L // 512   # 6 512-slices per r-block

_CACHE = {}


def _build():
    import concourse.bacc as bacc
    import concourse.bass as bass
    import concourse.mybir as mybir
    import concourse.tile as tile
    from concourse.masks import make_identity

    dt = mybir.dt
    f32 = dt.float32
    Alu = mybir.AluOpType
    Act = mybir.ActivationFunctionType

    nc = bacc.Bacc("TRN2", target_bir_lowering=False, debug=False, num_devices=B)

    def din(name, shape):
        return nc.declare_dram_parameter(name, list(shape), f32, isOutput=False)

    def dout(name, shape):
        return nc.declare_dram_parameter(name, list(shape), f32, isOutput=True)

    x_nat = din("x_nat", [L, H])
    x_T = din("x_T", [H, L])
    hg_pack = din("hg_pack", [128, HC])
    v_pack = din("v_pack", [128, HC])
    Wh = din("Wh", [H, 4])
    bh_row = din("bh_row", [1, 4])
    Wmats, brows = {}, {}
    for n in ["fs", "bs", "fr", "fg", "fx", "br", "bg", "bx"]:
        Wmats[n] = din("W_" + n, [H, H])
        brows[n] = din("b_" + n, [1, H])
    frelT_pack = din("frelT_pack", [128, HC * R])
    transeT = din("transeT", [TE, R])
    rproj_w = din("rproj_w", [TE, H])
    rproj_brow = din("rproj_brow", [1, H])

    o_probs = dout("o_probs", [4, L])
    o_masks = dout("o_masks", [2, L])
    o_sw = {"f": dout("o_f_sw", [L, H]), "b": dout("o_b_sw", [L, H])}
    o_A = {"f": dout("o_f_A", [L, R]), "b": dout("o_b_A", [L, R])}

    with tile.TileContext(nc) as tc, ExitStack() as ctx:
        consts = ctx.enter_context(tc.tile_pool(name="consts", bufs=1))
        wfull = ctx.enter_context(tc.tile_pool(name="wfull", bufs=9))
        sb = ctx.enter_context(tc.tile_pool(name="sb", bufs=2))
        sb1 = ctx.enter_context(tc.tile_pool(name="sb1", bufs=1))
        scan_p = ctx.enter_context(tc.tile_pool(name="scan", bufs=1))
        tanh_p = ctx.enter_context(tc.tile_pool(name="tanh", bufs=2))
        ps2 = ctx.enter_context(tc.tile_pool(name="ps2", bufs=2, space="PSUM"))
        ps_fv = ctx.enter_context(tc.tile_pool(name="ps_fv", bufs=1, space="PSUM"))

        def pst(shape):
            return ps2.tile(shape, f32, tag="pt", name="pt")

        # ---------------- constants ----------------
        ones_row = consts.tile([1, 512], f32)
        nc.vector.memset(ones_row, 1.0)
        iota_i32 = consts.tile([128, L], dt.int32)
        nc.gpsimd.iota(iota_i32, pattern=[[1, L]], base=0, channel_multiplier=0)
        iota_row = consts.tile([128, L], f32)
        nc.vector.tensor_copy(iota_row, iota_i32)            # [p, l] = l
        iotc_i32 = consts.tile([128, LT], dt.int32)
        nc.gpsimd.iota(iotc_i32, pattern=[[128, LT]], base=0, channel_multiplier=1)
        iota_col = consts.tile([128, LT], f32)
        nc.vector.tensor_copy(iota_col, iotc_i32)            # [p, t] = p + 128 t
        ident = consts.tile([128, 128], f32)
        make_identity(nc, ident)

        # ---------------- load small persistent inputs ----------------
        xT_sb = []
        for c in range(HC):
            t = sb1.tile([128, L], f32, tag=f"xT{c}")
            nc.sync.dma_start(out=t, in_=x_T[c * 128:(c + 1) * 128, :])
            xT_sb.append(t)
        xN_sb = []
        for t_i in range(LT):
            t = sb1.tile([128, H], f32, tag=f"xN{t_i}")
            nc.sync.dma_start(out=t, in_=x_nat[t_i * 128:(t_i + 1) * 128, :])
            xN_sb.append(t)
        hg_sb = sb1.tile([128, HC], f32, tag="hg")
        nc.sync.dma_start(out=hg_sb, in_=hg_pack[:, :])
        v_sb = sb1.tile([128, HC], f32, tag="v")
        nc.sync.dma_start(out=v_sb, in_=v_pack[:, :])
        frelT_sb = sb1.tile([128, HC * R], f32, tag="frelT")
        nc.sync.dma_start(out=frelT_sb, in_=frelT_pack[:, :])
        ones24 = ones_row[0:1, 0:R]

        # ---------------- brelT: projected TransE rel embs, [h,r] chunks ----
        transeT_sb = sb1.tile([TE, R], f32, tag="transeT")
        nc.sync.dma_start(out=transeT_sb, in_=transeT[:, :])
        rpb_sb = sb1.tile([1, H], f32, tag="rpb")
        nc.sync.dma_start(out=rpb_sb, in_=rproj_brow[:, :])
        brelT_sb = sb1.tile([128, HC * R], f32, tag="brelT")
        for m in range(HC):
            rw = wfull.tile([TE, 128], f32, tag="rpw")
            nc.sync.dma_start(out=rw, in_=rproj_w[:, m * 128:(m + 1) * 128])
            pt = pst([128, R])
            nc.tensor.matmul(pt, lhsT=rw, rhs=transeT_sb, start=True, stop=False)
            nc.tensor.matmul(pt, lhsT=rpb_sb[0:1, m * 128:(m + 1) * 128],
                             rhs=ones24, start=False, stop=True)
            nc.vector.tensor_copy(brelT_sb[:, m * R:(m + 1) * R], pt)

        # ---------------- head logits [4, L] ----------------
        bh_sb = sb1.tile([1, 4], f32, tag="bh")
        nc.sync.dma_start(out=bh_sb, in_=bh_row[:, :])
        logits_ps = pst([4, L])
        for c in range(HC):
            wt = wfull.tile([128, 4], f32, tag="wh")
            nc.sync.dma_start(out=wt, in_=Wh[c * 128:(c + 1) * 128, :])
            nc.tensor.matmul(logits_ps, lhsT=wt, rhs=xT_sb[c],
                             start=(c == 0), stop=False)
        nc.tensor.matmul(logits_ps, lhsT=bh_sb, rhs=ones_row[0:1, 0:L],
                         start=False, stop=True)
        logits_sb = sb1.tile([4, L], f32, tag="logits_sb")
        nc.vector.tensor_copy(logits_sb, logits_ps)
        # probs = 1/(1+exp(-l))  (exp set also hosts tanh -> one table load)
        expneg = sb1.tile([4, L], f32, tag="expneg")
        nc.scalar.activation(expneg, logits_sb, Act.Exp, scale=-1.0)
        nc.vector.tensor_single_scalar(expneg, expneg, 1.0, op=Alu.add)
        probs_sb = sb1.tile([4, L], f32, tag="probs")
        nc.vector.reciprocal(probs_sb, expneg)
        nc.sync.dma_start(out=o_probs[:, :], in_=probs_sb)

        lrow = []
        for k in range(4):
            lr = sb1.tile([1, L], f32, tag=f"lrow{k}")
            nc.sync.dma_start(out=lr, in_=logits_sb[k:k + 1, :])
            lrow.append(lr)

        for bi, br in enumerate(["f", "b"]):
            wW = {"f": ("fs", "fr", "fg", "fx"), "b": ("bs", "br", "bg", "bx")}[br]
            nWs, nWr, nWg, nWx = wW
            relT = frelT_sb if br == "f" else brelT_sb

            # ============ span pairing (vector land, [1, L] rows) ============
            s_m = scan_p.tile([1, L], f32, tag="s_m")
            e_m = scan_p.tile([1, L], f32, tag="e_m")
            nc.vector.tensor_single_scalar(s_m, lrow[2 * bi], 0.0, op=Alu.is_gt)
            nc.vector.tensor_single_scalar(e_m, lrow[2 * bi + 1], 0.0, op=Alu.is_gt)
            cs = scan_p.tile([1, L], f32, tag="cs")
            nc.vector.tensor_tensor_scan(cs, s_m, s_m, 0.0, op0=Alu.add, op1=Alu.bypass)
            ce = scan_p.tile([1, L], f32, tag="ce")
            nc.vector.tensor_tensor_scan(ce, e_m, e_m, 0.0, op0=Alu.add, op1=Alu.bypass)
            c_ex = scan_p.tile([1, L], f32, tag="c_ex")
            nc.vector.tensor_tensor(c_ex, ce, e_m, op=Alu.subtract)
            r_rk = scan_p.tile([1, L], f32, tag="r_rk")
            nc.vector.tensor_single_scalar(r_rk, cs, 1.0, op=Alu.subtract)
            # svec lanes (free-stacked): 0=j 1=slot 2=valid 3=re_m 4=nv
            svec_f = scan_p.tile([1, 5 * L], f32, tag="svec_f")
            svec = scan_p.tile([5, L], f32, tag="svec")

            def sv(k):
                return svec_f[0:1, k * L:(k + 1) * L]
            tdiff = scan_p.tile([1, L], f32, tag="tdiff")
            nc.vector.tensor_tensor(tdiff, c_ex, r_rk, op=Alu.subtract)
            u1 = scan_p.tile([1, L], f32, tag="u1")
            nc.vector.tensor_tensor(u1, tdiff, s_m, op=Alu.mult)
            u2 = scan_p.tile([1, L], f32, tag="u2")
            nc.vector.tensor_scalar(u2, s_m, 1.0, 1e9, op0=Alu.subtract, op1=Alu.mult)
            t_m = scan_p.tile([1, L], f32, tag="t_m")
            nc.vector.tensor_tensor(t_m, u1, u2, op=Alu.add)
            Mpm = scan_p.tile([1, L], f32, tag="Mpm")
            nc.vector.tensor_tensor_scan(Mpm, t_m, t_m, -1e9, op0=Alu.max, op1=Alu.bypass)
            nc.vector.tensor_tensor(sv(0), r_rk, Mpm, op=Alu.add)   # j
            jlt = scan_p.tile([1, L], f32, tag="jlt")
            nc.vector.tensor_scalar(jlt, sv(0), ce[0:1, L - 1:L], None,
                                    op0=Alu.is_lt)
            valid = scan_p.tile([1, L], f32, tag="valid")
            nc.vector.tensor_tensor(valid, jlt, s_m, op=Alu.mult)
            nc.vector.tensor_copy(sv(2), valid)
            cv = scan_p.tile([1, L], f32, tag="cv")
            nc.vector.tensor_tensor_scan(cv, valid, valid, 0.0, op0=Alu.add,
                                         op1=Alu.bypass)
            nc.vector.tensor_single_scalar(sv(1), cv, 1.0, op=Alu.subtract)
            # re_m = emask ? rank_e : -1 ;  rank_e = ce - 1
            rke = scan_p.tile([1, L], f32, tag="rke")
            nc.vector.tensor_single_scalar(rke, ce, 1.0, op=Alu.subtract)
            w1 = scan_p.tile([1, L], f32, tag="w1")
            nc.vector.tensor_tensor(w1, rke, e_m, op=Alu.mult)
            w2 = scan_p.tile([1, L], f32, tag="w2")
            nc.vector.tensor_single_scalar(w2, e_m, 1.0, op=Alu.subtract)
            nc.vector.tensor_tensor(sv(3), w1, w2, op=Alu.add)
            nv_ap = cv[0:1, L - 1:L]
            nc.vector.tensor_scalar(sv(4), ones_row[0:1, 0:L], nv_ap, None,
                                    op0=Alu.mult)
            nc.sync.dma_start(out=svec, in_=svec_f)
            # output mask row
            mrow = scan_p.tile([1, L], f32, tag="mrow")
            nc.vector.tensor_scalar(mrow, iota_row[0:1, :], nv_ap, None,
                                    op0=Alu.is_lt)
            nc.sync.dma_start(out=o_masks[bi:bi + 1, :], in_=mrow)

            # transpose svec halves -> per-partition scalars
            scal = []
            for hf in range(LT):
                pt = pst([128, 5])
                nc.tensor.transpose(pt, svec[0:5, hf * 128:(hf + 1) * 128],
                                    ident[0:5, 0:5])
                s = sb.tile([128, 5], f32, tag="scal")
                nc.vector.tensor_copy(s, pt)
                scal.append(s)   # cols: j slot valid re_m nv

            # broadcast j row across partitions: bc_j = ones^T @ j
            bc_j = pst([128, L])
            nc.tensor.matmul(bc_j, lhsT=ones_row[0:1, 0:128], rhs=svec[0:1, :],
                             start=True, stop=True)
            # ohT[p, i] = (re_m[p] == j_i)
            ohT = []
            for pc in range(LT):
                t = sb.tile([128, L], f32, tag="ohT")
                nc.vector.tensor_scalar(t, bc_j, scal[pc][:, 3:4], None,
                                        op0=Alu.is_equal)
                ohT.append(t)
            # pend per i-half + T1/Q matrices
            T1 = []
            Qm = []
            for hf in range(LT):
                pt = pst([128, 1])
                for pc in range(LT):
                    nc.tensor.matmul(pt, lhsT=ohT[pc][:, hf * 128:(hf + 1) * 128],
                                     rhs=iota_col[:, pc:pc + 1],
                                     start=(pc == 0), stop=(pc == LT - 1))
                pend = sb.tile([128, 1], f32, tag="pend")
                nc.vector.tensor_copy(pend, pt)
                t1 = sb.tile([128, L], f32, tag="T1")
                nc.vector.tensor_scalar(t1, iota_row, scal[hf][:, 1:2], None,
                                        op0=Alu.is_equal)
                nc.vector.tensor_scalar(t1, t1, scal[hf][:, 2:3], None, op0=Alu.mult)
                T1.append(t1)
                q = sb.tile([128, L], f32, tag="Q")
                nc.vector.tensor_scalar(q, iota_row, pend[:, 0:1], None,
                                        op0=Alu.is_equal)
                nc.vector.tensor_scalar(q, q, scal[hf][:, 2:3], None, op0=Alu.mult)
                Qm.append(q)
            # GT = T1 + Q^T @ T1   [t, k]
            GT = []
            for tt in range(LT):
                pt = pst([128, L])
                for ic in range(LT):
                    nc.tensor.matmul(pt, lhsT=Qm[ic][:, tt * 128:(tt + 1) * 128],
                                     rhs=T1[ic], start=(ic == 0), stop=(ic == LT - 1))
                g = sb.tile([128, L], f32, tag="GT")
                nc.vector.tensor_tensor(g, T1[tt], pt, op=Alu.add)
                GT.append(g)
            # padT[h, k] = 0.5 * tok^T @ GT : lhsT = x_nat tiles
            padT = []
            for c in range(HC):
                pt = pst([128, L])
                for tt in range(LT):
                    nc.tensor.matmul(pt, lhsT=xN_sb[tt][:, c * 128:(c + 1) * 128],
                                     rhs=GT[tt], start=(tt == 0), stop=(tt == LT - 1))
                p = sb.tile([128, L], f32, tag=f"padT{c}")
                nc.scalar.mul(p, pt, 0.5)
                padT.append(p)
            # f_mask per-partition cols
            fm_col = []
            for hf in range(LT):
                fm = sb.tile([128, 1], f32, tag="fm")
                nc.vector.tensor_scalar(fm, iota_col[:, hf:hf + 1],
                                        scal[hf][:, 4:5], None, op0=Alu.is_lt)
                fm_col.append(fm)
            # sw = (pad @ W_s + b_s) * mask : kt-outer, W_s fully resident
            Ws_t = []
            for c in range(HC):
                wt = wfull.tile([128, H], f32, tag="w")
                nc.sync.dma_start(out=wt, in_=Wmats[nWs][c * 128:(c + 1) * 128, :])
                Ws_t.append(wt)
            bs_sb = sb.tile([1, H], f32, tag="brow")
            nc.sync.dma_start(out=bs_sb, in_=brows[nWs][:, :])
            for kt in range(LT):
                for n0 in range(2):
                    nsl = slice(n0 * 512, min(H, (n0 + 1) * 512))
                    nn = nsl.stop - nsl.start
                    pt = pst([128, 512])
                    for c in range(HC):
                        nc.tensor.matmul(pt[:, 0:nn],
                                         lhsT=padT[c][:, kt * 128:(kt + 1) * 128],
                                         rhs=Ws_t[c][:, nsl], start=(c == 0),
                                         stop=False)
                    nc.tensor.matmul(pt[:, 0:nn], lhsT=ones_row[0:1, 0:128],
                                     rhs=bs_sb[0:1, nsl], start=False, stop=True)
                    swt = sb.tile([128, 512], f32, tag="swt")
                    nc.vector.tensor_scalar(swt[:, 0:nn], pt[:, 0:nn],
                                            fm_col[kt][:, 0:1], None, op0=Alu.mult)
                    nc.sync.dma_start(out=o_sw[br][kt * 128:(kt + 1) * 128, nsl],
                                      in_=swt[:, 0:nn])

            # ============ attention path ============
            # wg_row = h_g @ W_g + b_g + b_r   [1, H]
            bg_sb = sb.tile([1, H], f32, tag="brow")
            nc.sync.dma_start(out=bg_sb, in_=brows[nWg][:, :])
            br_sb = sb.tile([1, H], f32, tag="brow")
            nc.sync.dma_start(out=br_sb, in_=brows[nWr][:, :])
            Wg_t = []
            for c in range(HC):
                wt = wfull.tile([128, H], f32, tag="w")
                nc.sync.dma_start(out=wt, in_=Wmats[nWg][c * 128:(c + 1) * 128, :])
                Wg_t.append(wt)
            wg_row = sb.tile([1, H], f32, tag="wg_row")
            for n0 in range(2):
                nsl = slice(n0 * 512, min(H, (n0 + 1) * 512))
                nn = nsl.stop - nsl.start
                wg_ps = pst([1, 512])
                for c in range(HC):
                    nc.tensor.matmul(wg_ps[0:1, 0:nn], lhsT=hg_sb[:, c:c + 1],
                                     rhs=Wg_t[c][:, nsl], start=(c == 0), stop=False)
                nc.tensor.matmul(wg_ps[0:1, 0:nn], lhsT=ones_row[0:1, 0:1],
                                 rhs=bg_sb[0:1, nsl], start=False, stop=False)
                nc.tensor.matmul(wg_ps[0:1, 0:nn], lhsT=ones_row[0:1, 0:1],
                                 rhs=br_sb[0:1, nsl], start=False, stop=True)
                nc.vector.tensor_copy(wg_row[0:1, nsl], wg_ps[0:1, 0:nn])

            # AT[h_out, r] = (rel @ W_r)^T + wg_row^T x ones : m-outer
            Wr_t = []
            for c in range(HC):
                wt = wfull.tile([128, H], f32, tag="w")
                nc.sync.dma_start(out=wt, in_=Wmats[nWr][c * 128:(c + 1) * 128, :])
                Wr_t.append(wt)
            AT_sb = sb1.tile([128, HC * R], f32, tag="AT")
            for m in range(HC):
                pt = pst([128, R])
                for c in range(HC):
                    nc.tensor.matmul(pt, lhsT=Wr_t[c][:, m * 128:(m + 1) * 128],
                                     rhs=relT[:, c * R:(c + 1) * R],
                                     start=(c == 0), stop=False)
                nc.tensor.matmul(pt, lhsT=wg_row[0:1, m * 128:(m + 1) * 128],
                                 rhs=ones24, start=False, stop=True)
                nc.vector.tensor_copy(AT_sb[:, m * R:(m + 1) * R], pt)

            # wxT[h_out, l] chunks: m-outer, W_x fully resident
            Wx_t = []
            for c in range(HC):
                wt = wfull.tile([128, H], f32, tag="w")
                nc.sync.dma_start(out=wt, in_=Wmats[nWx][c * 128:(c + 1) * 128, :])
                Wx_t.append(wt)
            bx_sb = sb.tile([1, H], f32, tag="brow")
            nc.sync.dma_start(out=bx_sb, in_=brows[nWx][:, :])
            wxT = []
            for m in range(HC):
                pt = pst([128, L])
                for c in range(HC):
                    nc.tensor.matmul(pt, lhsT=Wx_t[c][:, m * 128:(m + 1) * 128],
                                     rhs=xT_sb[c], start=(c == 0), stop=False)
                nc.tensor.matmul(pt, lhsT=bx_sb[0:1, m * 128:(m + 1) * 128],
                                 rhs=ones_row[0:1, 0:L], start=False, stop=True)
                w = sb1.tile([128, L], f32, tag=f"wxT{m}")
                nc.vector.tensor_copy(w, pt)
                wxT.append(w)

            # tanh + V-dot per r-block
            fv_sb = sb1.tile([R, L], f32, tag="fv")
            for rb in range(RB):
                fv_ps = ps_fv.tile([1, RPB * L], f32, tag="fv_ps")
                for c in range(HC):
                    ti = tanh_p.tile([128, RPB * L], f32, tag="ti")
                    for r in range(RPB):
                        nc.vector.tensor_scalar(
                            ti[:, r * L:(r + 1) * L], wxT[c],
                            AT_sb[:, c * R + rb * RPB + r:c * R + rb * RPB + r + 1],
                            None, op0=Alu.add)
                    to = tanh_p.tile([128, RPB * L], f32, tag="to")
                    nc.scalar.activation(to, ti, Act.Tanh)
                    for s in range(SLC):
                        nc.tensor.matmul(fv_ps[0:1, s * 512:(s + 1) * 512],
                                         lhsT=v_sb[:, c:c + 1],
                                         rhs=to[:, s * 512:(s + 1) * 512],
                                         start=(c == 0), stop=(c == HC - 1))
                fv_row = sb.tile([1, RPB * L], f32, tag="fv_row")
                nc.vector.tensor_copy(fv_row, fv_ps)
                nc.sync.dma_start(out=fv_sb[rb * RPB:(rb + 1) * RPB, :],
                                  in_=fv_row)
            # softmax over l per r
            mxn = sb.tile([R, 1], f32, tag="mxn")
            nc.vector.tensor_reduce(mxn, fv_sb, axis=mybir.AxisListType.X,
                                    op=Alu.max, negate=True)
            esb = sb.tile([R, L], f32, tag="esb")
            ssum = sb.tile([R, 1], f32, tag="ssum")
            nc.scalar.activation(esb, fv_sb, Act.Exp, bias=mxn[:, 0:1],
                                 accum_out=ssum[:, 0:1])
            sinv = sb.tile([R, 1], f32, tag="sinv")
            nc.vector.reciprocal(sinv, ssum)
            att = sb.tile([R, L], f32, tag="att")
            nc.vector.tensor_scalar(att, esb, sinv[:, 0:1], None, op0=Alu.mult)
            for hf in range(LT):
                pt = pst([128, R])
                nc.tensor.transpose(pt, att[0:R, hf * 128:(hf + 1) * 128],
                                    ident[0:R, 0:R])
                ao = sb.tile([128, R], f32, tag="ao")
                nc.vector.tensor_copy(ao, pt)
                nc.sync.dma_start(out=o_A[br][hf * 128:(hf + 1) * 128, :], in_=ao)

    nc.compile()
    return nc


def _prep_inputs(h_gs, embs, params):
    """Per-core input dicts. Host does layout only (transpose/reshape/pack)."""
    f32 = np.float32
    P = params

    def arr(x):
        return np.ascontiguousarray(np.asarray(x, dtype=f32))

    Wh = np.concatenate([arr(P[n]["w"]) for n in
                         ["f_start_sub", "f_end_sub", "b_start_obj", "b_end_obj"]],
                        axis=1)                                    # [H,4]
    bh = np.concatenate([arr(P[n]["b"]) for n in
                         ["f_start_sub", "f_end_sub", "b_start_obj", "b_end_obj"]]
                        ).reshape(1, 4)
    common = {"Wh": Wh, "bh_row": bh}
    for short, name in [("fs", "f_W_s"), ("bs", "b_W_s"), ("fr", "f_W_r"),
                        ("fg", "f_W_g"), ("fx", "f_W_x"), ("br", "b_W_r"),
                        ("bg", "b_W_g"), ("bx", "b_W_x")]:
        common["W_" + short] = arr(P[name]["w"])
        common["b_" + short] = arr(P[name]["b"]).reshape(1, H)
    frelT = arr(P["rel_embs"]).T                                   # [H, R]
    common["frelT_pack"] = np.ascontiguousarray(
        frelT.reshape(HC, 128, R).transpose(1, 0, 2).reshape(128, HC * R))
    common["transeT"] = np.ascontiguousarray(arr(P["rel_transe_embs"]).T)
    common["rproj_w"] = arr(P["r_proj"]["w"])
    common["rproj_brow"] = arr(P["r_proj"]["b"]).reshape(1, H)
    common["v_pack"] = np.ascontiguousarray(
        arr(P["V"]["w"]).reshape(HC, 128).T)

    maps = []
    for b in range(B):
        m = dict(common)
        xb = arr(embs[b])
        m["x_nat"] = xb
        m["x_T"] = np.ascontiguousarray(xb.T)
        m["hg_pack"] = np.ascontiguousarray(arr(h_gs[b]).reshape(HC, 128).T)
        maps.append(m)
    return maps


def kernel(h_gs, embs, params):
    from concourse.bass_utils import run_bass_kernel_spmd

    if "nc" not in _CACHE:
        _CACHE["nc"] = _build()
    nc = _CACHE["nc"]
    maps = _prep_inputs(h_gs, embs, params)
    res = run_bass_kernel_spmd(nc, maps, list(range(B))).results

    f_ss = np.stack([res[b]["o_probs"][0] for b in range(B)])
    f_se = np.stack([res[b]["o_probs"][1] for b in range(B)])
    b_os = np.stack([res[b]["o_probs"][2] for b in range(B)])
    b_oe = np.stack([res[b]["o_probs"][3] for b in range(B)])
    f_sw = np.stack([res[b]["o_f_sw"] for b in range(B)])
    b_sw = np.stack([res[b]["o_b_sw"] for b in range(B)])
    f_mask = np.stack([res[b]["o_masks"][0] for b in range(B)]) > 0.5
    b_mask = np.stack([res[b]["o_masks"][1] for b in range(B)]) > 0.5
    f_A = np.stack([res[b]["o_f_A"] for b in range(B)])[:, :, :, None]
    b_A = np.stack([res[b]["o_b_A"] for b in range(B)])[:, :, :, None]
    return (f_ss, f_se, b_os, b_oe, f_sw, f_mask, b_sw, b_mask, f_A, b_A)
